# revision 10
# baseline (speedup 1.0000x reference)
"""Trainium2 Bass kernel for nn_CrossAttention_17033840296537.

Full-input contract: kernel(**inputs) takes the unsharded tensors as in
reference.setup_inputs() and returns the full [8, 2048, 512] output.

Sharding: data-parallel over batch B=8 across the 8 NeuronCores (one
batch element per core). Weights are replicated.

Per-core design (fp8 DoubleRow for the dominant matmuls, f32 PSUM):
  prologue (everything SBUF-resident, no DRAM scratch):
    qk_w^T -> qkwT (bf16)
    q^T, k^T via PE transposes, projected to qhT/khT [512hd, 2048] bf16
    v cast to vtp fp8e4 pair tiles (lhsT for attn@V, 2 s-blocks each)
    M_h = wv_h^T @ fc_w[:,h]^T  [512c, 512o] bf16 per head -- merges the
        v-projection and the output fc into ONE matmul stage downstream.
  main loop, flat pipeline over 256 (j, h, si) pair-jobs:
    scores^T[s,q] = khT[h] slices^T @ qhT[h]   (K=64, bf16, tile_position
        by head parity)
    ptp = exp(scores*0.125 + mask_bias - C)  fp8e5 pair tile,
        UNNORMALIZED, C=2.5 keeps exp inside e5m2 range; the shift
        cancels exactly in the softmax division.
    t1[c,q]  += vtp^T ptp   (DoubleRow fp8: K=256 per instr, 2x rate)
    r[q]     += ones^T ptp  (softmax denominator, DoubleRow too)
    fc partial fp[q,o] = sum_cb t1s[cb]^T M_h[cb]  (bf16; fp8 here costs
        too much accuracy: rel-err 1.9e-2 vs the 2e-2 gate)
    facc[q,o] = fp * (1/r)[q] + facc   (ONE fused DVE op; 1/r arrives as
        a per-partition column via 4 tiny K=1 matmuls + reciprocal)
  fc/facc/epilogue work of iteration i is emitted interleaved into
  iteration i+1's score/t1 stream so the PE never drains.
  epilogue per j: += idt, LayerNorm (Sqrt on scalar, batched), -> out.
"""

import numpy as np

import concourse.bass as bass
import concourse.tile as tile
from concourse import mybir
from concourse.bass import ds
from concourse.masks import make_identity

F32 = mybir.dt.float32
BF = mybir.dt.bfloat16
E4 = mybir.dt.float8e4
E5 = mybir.dt.float8e5
I32 = mybir.dt.int32
AF = mybir.ActivationFunctionType
ALU = mybir.AluOpType
DRMODE = mybir.MatmulPerfMode.DoubleRow

B = 8
NQ = NS = 2048
DIM = 512          # input channel dim (DIM_K == DIM_V == 512)
N_HEAD = 8
D_K = 64
D_V = 512
HD = N_HEAD * D_V  # 4096 concat dim
P = 128
C_SHIFT = 2.5      # exp shift: max logit ~12.5, e5m2 max 57344=e^10.96


def _emit(tc: tile.TileContext, io: dict):
    nc = tc.nc
    q, k, v, mask, idt = io["q"], io["k"], io["v"], io["mask"], io["idt"]
    qk_w, v_w, fc_w = io["qk_w"], io["v_w"], io["fc_w"]
    fc_b, ln_g, ln_b = io["fc_b"], io["ln_g"], io["ln_b"]
    out = io["out"]

    cpool_cm = tc.tile_pool(name="cpool", bufs=1)
    rpool_cm = tc.tile_pool(name="rpool", bufs=1)
    cpool = cpool_cm.__enter__()
    rpool = rpool_cm.__enter__()

    # ---- constants ----
    ident = cpool.tile([P, P], F32, name="ident")
    make_identity(nc, ident)
    # 2x32 block of ones: DoubleRow stationary for the softmax-denominator
    # matmul (a [2,1]-shaped stationary fails the walrus ISA check; 32-wide
    # mirrors the minimum tile_size column group)
    ones_f = cpool.tile([P, 64], F32, name="ones_f")
    nc.vector.memset(ones_f, 1.0)
    ones2 = cpool.tile([P, 64], E4, name="ones2")
    nc.vector.tensor_copy(out=ones2, in_=ones_f)
    one11 = cpool.tile([1, 1], F32, name="one11")
    nc.vector.memset(one11, 1.0)
    eps_t = cpool.tile([P, 1], F32, name="eps_t")
    nc.vector.memset(eps_t, 1e-5)

    def bcast_row(name, src):  # [512] dram -> [128, 512] sbuf (rows identical)
        bc = cpool.tile([P, D_V], F32, name=name + "_bc")
        src_b = bass.AP(tensor=src.tensor, offset=src.offset,
                        ap=[[0, P]] + list(src.ap))
        nc.gpsimd.dma_start(out=bc, in_=src_b)
        return bc

    fcb_bc = bcast_row("fcb", fc_b)
    lng_bc = bcast_row("lng", ln_g)
    lnb_bc = bcast_row("lnb", ln_b)

    mask_i = cpool.tile([P, 16], I32, name="mask_i")
    nc.gpsimd.dma_start(out=mask_i, in_=mask.rearrange("(a p) -> p a", p=P))
    mask_b = cpool.tile([P, 16], F32, name="mask_b")
    nc.vector.tensor_copy(out=mask_b, in_=mask_i)  # int32 -> f32 cast
    nc.scalar.mul(mask_b, mask_b, -10000.0)
    negc = cpool.tile([P, 1], F32, name="negc")
    nc.vector.memset(negc, -C_SHIFT)
    nc.vector.tensor_scalar(out=mask_b, in0=mask_b, scalar1=negc,
                            scalar2=None, op0=ALU.add)

    # ---- residents ----
    # vtp[si] holds s-blocks (2si, 2si+1) side by side as the DoubleRow
    # stationary operand for attn@V
    vtp = [rpool.tile([P, 2 * DIM], E4, name=f"vtp{si}") for si in range(8)]
    qhT = [rpool.tile([P, NQ], BF, name=f"qhT{mb}") for mb in range(4)]
    khT = [rpool.tile([P, NS], BF, name=f"khT{mb}") for mb in range(4)]
    Msb = [[rpool.tile([P, D_V], BF, name=f"M{h}_{cb}") for cb in range(4)]
           for h in range(N_HEAD)]
    facc = [rpool.tile([P, D_V], F32, name=f"facc{i}") for i in range(16)]

    # ================= prologue =================
    # DMA: 512-row blocks as single [128, 2048] rearranged transfers.
    # sync queue:   qk_w, q, k           (feeds the projection pipeline)
    # gpsimd queue: fc_w/v_w per head, v (weight/value path), then casts
    def blk(src, r0, cols=512, c0=0):
        # 512 DRAM rows (cols c0:c0+cols) -> [128 p, 4*cols] view where
        # element (p, a*cols + c) = src[r0 + a*128 + p, c0 + c]
        rstr = src.ap[0][0]
        return bass.AP(tensor=src.tensor,
                       offset=src.offset + r0 * rstr + c0,
                       ap=[[rstr, P], [P * rstr, 4], [1, cols]])

    with (
        tc.tile_pool(name="pstage", bufs=1) as pstage,
        tc.tile_pool(name="ppsum", bufs=1, space="PSUM") as pp,
    ):
        # ---- all DMA triggers up front ----
        # sync queue in priority order (q,k feed the PE first, then the
        # per-head weights) so transfers don't compete for HBM bandwidth;
        # v rides the gpsimd queue concurrently (small, needed mid-phase).
        qkw_stg = pstage.tile([P, 2048], F32, name="qkw_stg", tag="qld",
                              bufs=3)
        nc.sync.dma_start(out=qkw_stg, in_=blk(qk_w, 0))
        q0_stg = pstage.tile([P, 2048], F32, name="q0_stg", tag="qld",
                             bufs=3)
        nc.sync.dma_start(out=q0_stg, in_=blk(q, 0))
        qk_stg = [q0_stg]
        for src, sname in ((q, "q"), (k, "k")):
            for j2 in range(4):
                if src is q and j2 == 0:
                    continue
                st = pstage.tile([P, 2048], F32, name=f"{sname}stg{j2}",
                                 tag="qld", bufs=3)
                nc.sync.dma_start(out=st, in_=blk(src, j2 * 512))
                qk_stg.append(st)
        vstg = []
        for c4 in range(4):
            st = pstage.tile([P, 2048], F32, name=f"vstg{c4}", tag="vstg",
                             bufs=2)
            nc.gpsimd.dma_start(out=st, in_=blk(v, c4 * 512))
            vstg.append(st)
        wstg = []
        for h in range(N_HEAD):
            ft = pstage.tile([P, 2048], F32, name=f"fstg{h}", tag="wstg",
                             bufs=3)
            nc.sync.dma_start(out=ft, in_=blk(fc_w, 0, c0=h * 512))
            vw_raw = pstage.tile([P, 2048], F32, name=f"vwstg{h}",
                                 tag="wstg", bufs=3)
            nc.sync.dma_start(out=vw_raw, in_=blk(v_w, h * 512))
            wstg.append((ft, vw_raw))

        # v_w casts per head, split gpsimd/scalar so neither queue exceeds
        # the PE's ~5.1us per-head M-build pace
        vwb = []
        for h in range(N_HEAD):
            vws = []
            for i in range(4):
                vb = pstage.tile([P, 512], BF, name=f"vwb{h}_{i}",
                                 tag="vwb", bufs=5)
                eng = nc.gpsimd if i < 2 else nc.scalar
                if eng is nc.gpsimd:
                    nc.gpsimd.tensor_copy(out=vb,
                                          in_=wstg[h][1][:, ds(i * 512, 512)])
                else:
                    nc.scalar.copy(out=vb,
                                   in_=wstg[h][1][:, ds(i * 512, 512)])
                vws.append(vb)
            vwb.append(vws)

        # ---- qk_w^T -> qkwT bf16 ----
        qkwT = []
        for cb in range(4):
            tp = pp.tile([P, 512], F32, name=f"tpw{cb}", tag="tp", bufs=3)
            for rb in range(4):
                nc.tensor.transpose(tp[:, ds(rb * P, P)],
                                    qkw_stg[:, ds(rb * 512 + cb * P, P)],
                                    ident)
            qw = pstage.tile([P, 512], BF, name=f"qkwT{cb}", tag=f"qkwT{cb}")
            nc.vector.tensor_copy(out=qw, in_=tp)
            qkwT.append(qw)

        # ---- q, k: transpose + project -> qhT/khT bf16 (SBUF resident) ----
        # transposes run one chunk ahead of the projection waves so the
        # DVE qTc copies are always hidden behind PE work
        def emit_qkT(ci):
            stg = qk_stg[ci]
            qTc = []
            for cb in range(4):
                tp = pp.tile([P, 512], F32, name=f"tpq{ci}_{cb}",
                             tag="tp", bufs=3)
                for qb in range(4):
                    nc.tensor.transpose(
                        tp[:, ds(qb * P, P)],
                        stg[:, ds(qb * 512 + cb * P, P)], ident)
                qc = pstage.tile([P, 512], BF, name=f"qTc{ci}_{cb}",
                                 tag="qTc", bufs=8)
                nc.vector.tensor_copy(out=qc, in_=tp)
                qTc.append(qc)
            return qTc

        def emit_proj(ci, qTc):
            dstT = qhT if ci < 4 else khT
            j2 = ci % 4
            # cb-outer so each matmul wave depends on only one qTc copy
            prs = [pp.tile([P, 512], F32, name=f"pr{ci}_{mb}",
                           tag="pr", bufs=4) for mb in range(4)]
            for cb in range(4):
                for mb in range(4):
                    nc.tensor.matmul(prs[mb],
                                     lhsT=qkwT[cb][:, ds(mb * P, P)],
                                     rhs=qTc[cb],
                                     start=(cb == 0), stop=(cb == 3))
            for mb in range(4):
                nc.scalar.copy(out=dstT[mb][:, ds(j2 * 512, 512)],
                               in_=prs[mb])

        qTc_cur = emit_qkT(0)
        for ci in range(8):
            qTc_nxt = emit_qkT(ci + 1) if ci < 7 else None
            emit_proj(ci, qTc_cur)
            qTc_cur = qTc_nxt

        # ---- per-head merged projection M_h = wv_h^T @ fc_w[:,h]^T ----
        # fwT transposes run one head ahead of the M matmul waves
        def emit_fwT(h):
            fstg = wstg[h][0]
            fwT = []
            for db in range(4):
                tp = pp.tile([P, 512], F32, name=f"tpf{h}_{db}",
                             tag="tp", bufs=3)
                for rb in range(4):
                    nc.tensor.transpose(
                        tp[:, ds(rb * P, P)],
                        fstg[:, ds(rb * 512 + db * P, P)], ident)
                fw = pstage.tile([P, 512], BF, name=f"fwT{h}_{db}",
                                 tag="fwT", bufs=8)
                nc.vector.tensor_copy(out=fw, in_=tp)
                fwT.append(fw)
            return fwT

        def emit_M(h, fwT):
            # two v casts per head on DVE -- spreads them so vtp is ready
            # just before the main loop consumes it
            for half, sb in enumerate((2 * h, 2 * h + 1)):
                nc.vector.tensor_copy(
                    out=vtp[h][:, ds(half * 512, 512)],
                    in_=vstg[sb // 4][:, ds((sb % 4) * 512, 512)])
            # i-outer so each matmul wave depends on only one fwT copy
            prs = [pp.tile([P, 512], F32, name=f"prM{h}_{cb}",
                           tag="pr", bufs=4) for cb in range(4)]
            for i in range(4):
                for cb in range(4):
                    nc.tensor.matmul(prs[cb],
                                     lhsT=vwb[h][i][:, ds(cb * P, P)],
                                     rhs=fwT[i],
                                     start=(i == 0), stop=(i == 3))
            for cb in range(4):
                nc.scalar.copy(out=Msb[h][cb], in_=prs[cb])

        fwT_cur = emit_fwT(0)
        for h in range(N_HEAD):
            fwT_nxt = emit_fwT(h + 1) if h < 7 else None
            emit_M(h, fwT_cur)
            fwT_cur = fwT_nxt

    # ================= main =================
    with (
        tc.tile_pool(name="ms", bufs=1) as ms,
        tc.tile_pool(name="mp", bufs=1, space="PSUM") as mp,
    ):
        iters = [(j, h) for j in range(4) for h in range(8)]
        idt_tiles = {}   # j -> tile
        prev = None      # dict carrying previous iteration's state
        ep_pending = []  # j values whose epilogue is ready to emit

        def emit_idt_loads(j):
            it = ms.tile([P, 2048], F32, name=f"idt{j}", tag="idt", bufs=1)
            nc.sync.dma_start(out=it, in_=blk(idt, j * 512))
            idt_tiles[j] = it

        def emit_rcol(pv, idx):
            # previous iteration's softmax sums [1,512] -> per-partition
            # column [128,4] + reciprocal; rides the "fp" PSUM bank.
            rcolt = mp.tile([P, 512], F32, name=f"rcol{idx}", tag="fp",
                            bufs=1)
            for qb in range(4):
                nc.tensor.matmul(rcolt[:, ds(qb, 1)],
                                 lhsT=pv["rs"][0:1, ds(qb * P, P)],
                                 rhs=one11, start=True, stop=True)
            rinv = ms.tile([P, 4], F32, name=f"rinv{idx}", tag="rinv", bufs=2)
            nc.vector.reciprocal(rinv, rcolt[:, 0:4])
            pv["rinv"] = rinv

        def emit_fc_group(pv, qb, idx):
            fpt = mp.tile([P, 512], F32, name=f"fp{idx}_{qb}", tag="fp",
                          bufs=1)
            for cb in range(4):
                nc.tensor.matmul(fpt,
                                 lhsT=pv["t1s"][cb][:, ds(qb * P, P)],
                                 rhs=Msb[pv["h"]][cb],
                                 start=(cb == 0), stop=(cb == 3))
            i16 = pv["j"] * 4 + qb
            in1 = fcb_bc if pv["h"] == 0 else facc[i16]
            nc.vector.scalar_tensor_tensor(out=facc[i16], in0=fpt,
                                           scalar=pv["rinv"][:, ds(qb, 1)],
                                           in1=in1,
                                           op0=ALU.mult, op1=ALU.add)
            if pv["h"] == 7 and qb == 3:
                ep_pending.append(pv["j"])

        def emit_epilogue(j):
            # residual + LayerNorm, in place on the facc tiles
            xts, mvs = [], []
            for qb in range(4):
                i16 = j * 4 + qb
                xt = facc[i16]
                nc.vector.tensor_add(xt, xt,
                                     idt_tiles[j][:, ds(qb * 512, 512)])
                st = ms.tile([P, 6], F32, name=f"st{i16}", tag="st", bufs=4)
                nc.vector.bn_stats(out=st, in_=xt)
                mv = ms.tile([P, 2], F32, name=f"mv{i16}", tag="mv", bufs=4)
                nc.vector.bn_aggr(out=mv, in_=st)
                xts.append(xt)
                mvs.append(mv)
            sds = []
            for qb in range(4):  # batched so the scalar engine swaps its
                i16 = j * 4 + qb  # activation table Exp->Sqrt only once
                sd = ms.tile([P, 1], F32, name=f"sd{i16}", tag="sd", bufs=4)
                nc.scalar.activation(sd, mvs[qb][:, 1:2], AF.Sqrt,
                                     bias=eps_t)
                sds.append(sd)
            rstds = []
            for qb in range(4):
                i16 = j * 4 + qb
                rstd = ms.tile([P, 1], F32, name=f"rstd{i16}", tag="rstd",
                               bufs=4)
                nc.vector.reciprocal(rstd, sds[qb])
                rstds.append(rstd)
            for qb in range(4):
                i16 = j * 4 + qb
                xt = xts[qb]
                nc.vector.tensor_scalar(out=xt, in0=xt,
                                        scalar1=mvs[qb][:, 0:1],
                                        scalar2=rstds[qb],
                                        op0=ALU.subtract, op1=ALU.mult)
                nc.vector.tensor_mul(xt, xt, lng_bc)
                nc.vector.tensor_add(xt, xt, lnb_bc)
                nc.sync.dma_start(out=out[ds(i16 * P, P), :], in_=xt)

        # ---- flat pipeline over 256 pair-jobs (32 iters x 8 si) ----
        state = {}  # per-iter psum tiles, created at si == 0

        def emit_pair(n):
            idx, si = n // 8, n % 8
            j, h = iters[idx]
            par = h % 2
            tnum = h // 2
            po = par * D_K
            tiles = []
            for sb in (2 * si, 2 * si + 1):
                sct = mp.tile([P, 512], F32, name=f"sc{idx}_{sb}",
                              tag="sc", bufs=2)
                nc.tensor.matmul(sct,
                                 lhsT=khT[tnum][po:po + D_K, ds(sb * P, P)],
                                 rhs=qhT[tnum][po:po + D_K, ds(j * 512, 512)],
                                 start=True, stop=True,
                                 tile_position=(po, 0))
                tiles.append(sct)
            return tiles

        def emit_exp(n, pair):
            idx, si = n // 8, n % 8
            ptp = ms.tile([P, 1024], E5, name=f"pt{idx}_{si}", tag="pt",
                          bufs=3)
            for half in range(2):
                sb = 2 * si + half
                nc.scalar.activation(ptp[:, ds(half * 512, 512)], pair[half],
                                     AF.Exp, bias=mask_b[:, ds(sb, 1)],
                                     scale=0.125)
            return ptp

        def emit_t1_dr(n, ptp):
            idx, si = n // 8, n % 8
            if si == 0:
                state["t1"] = mp.tile([P, 4 * 512], F32, name=f"t1_{idx}",
                                      tag="t1", bufs=1)
                state["r2a"] = mp.tile([32, 512], F32, name=f"r2a_{idx}",
                                       tag="r2a", bufs=1)
            t1, r2a = state["t1"], state["r2a"]
            rhs = ptp.rearrange("p (two n) -> p two n", two=2)
            for cb in range(4):
                lhsT = bass.AP(tensor=vtp[si].tensor,
                               offset=vtp[si].offset + cb * P,
                               ap=[vtp[si].ap[0], [512, 2], [1, P]])
                nc.tensor.matmul(t1[:, ds(cb * 512, 512)], lhsT=lhsT,
                                 rhs=rhs, start=(si == 0), stop=(si == 7),
                                 perf_mode=DRMODE)
            lones = bass.AP(tensor=ones2.tensor, offset=ones2.offset,
                            ap=[ones2.ap[0], [32, 2], [1, 32]])
            nc.tensor.matmul(r2a, lhsT=lones, rhs=rhs,
                             start=(si == 0), stop=(si == 7),
                             perf_mode=DRMODE)

        def emit_iter_end(idx):
            # softmax sums out, t1 -> SBUF bf16 split across scalar+vector
            # (gpsimd cannot read PSUM) to minimize t1 PSUM free latency
            nonlocal prev
            j, h = iters[idx]
            t1, r2a = state["t1"], state["r2a"]
            rs = ms.tile([1, 512], F32, name=f"rs{idx}", tag="rs", bufs=2)
            nc.vector.tensor_copy(out=rs, in_=r2a[0:1, :])
            t1s = [ms.tile([P, 512], BF, name=f"t1s{idx}_{cb}", tag="t1s",
                           bufs=8) for cb in range(4)]
            nc.scalar.copy(out=t1s[0], in_=t1[:, ds(0, 512)])
            nc.scalar.copy(out=t1s[1], in_=t1[:, ds(512, 512)])
            nc.vector.tensor_copy(out=t1s[2], in_=t1[:, ds(1024, 512)])
            nc.vector.tensor_copy(out=t1s[3], in_=t1[:, ds(1536, 512)])
            prev = {"j": j, "h": h, "t1s": t1s, "rs": rs, "rinv": None}

        def emit_extras(n):
            idx, si = n // 8, n % 8
            j, h = iters[idx]
            if prev is not None and si == 0:
                emit_rcol(prev, idx)
            if prev is not None and 1 <= si <= 4:
                emit_fc_group(prev, si - 1, idx)
            if si == 5:
                if h == 5:
                    emit_idt_loads(j)
                if ep_pending and h >= 1:
                    emit_epilogue(ep_pending.pop(0))

        # Two-pair-deep exp pipeline: the scalar engine's exp (~0.7us per
        # half) gets two full PE steps of slack, so the score matmuls'
        # WAR on the sc PSUM buffers (bufs=2) never stalls the PE.
        NJOBS = 256
        ptps = {}
        for m in range(2):
            ptps[m] = emit_exp(m, emit_pair(m))
        for n in range(NJOBS):
            if n + 2 < NJOBS:
                pair_nxt = emit_pair(n + 2)
            emit_t1_dr(n, ptps.pop(n))
            if n % 8 == 7:
                emit_iter_end(n // 8)
            if n + 2 < NJOBS:
                ptps[n + 2] = emit_exp(n + 2, pair_nxt)
            emit_extras(n)

        # drain the last iteration + epilogue for j=3
        emit_rcol(prev, 32)
        for qb in range(4):
            emit_fc_group(prev, qb, 32)
        while ep_pending:
            emit_epilogue(ep_pending.pop(0))

    rpool_cm.__exit__(None, None, None)
    cpool_cm.__exit__(None, None, None)


def build_nc():
    from concourse import bacc
    nc = bacc.Bacc("TRN2", target_bir_lowering=False, debug=False)
    io = {}
    io["q"] = nc.dram_tensor("q", [NQ, DIM], F32, kind="ExternalInput").ap()
    io["k"] = nc.dram_tensor("k", [NS, DIM], F32, kind="ExternalInput").ap()
    io["v"] = nc.dram_tensor("v", [NS, DIM], F32, kind="ExternalInput").ap()
    io["mask"] = nc.dram_tensor("mask", [NS], I32, kind="ExternalInput").ap()
    io["idt"] = nc.dram_tensor("idt", [NQ, D_V], F32, kind="ExternalInput").ap()
    io["qk_w"] = nc.dram_tensor("qk_w", [512, DIM], F32, kind="ExternalInput").ap()
    io["v_w"] = nc.dram_tensor("v_w", [HD, DIM], F32, kind="ExternalInput").ap()
    io["fc_w"] = nc.dram_tensor("fc_w", [D_V, HD], F32, kind="ExternalInput").ap()
    io["fc_b"] = nc.dram_tensor("fc_b", [D_V], F32, kind="ExternalInput").ap()
    io["ln_g"] = nc.dram_tensor("ln_g", [D_V], F32, kind="ExternalInput").ap()
    io["ln_b"] = nc.dram_tensor("ln_b", [D_V], F32, kind="ExternalInput").ap()
    io["out"] = nc.dram_tensor("out", [NQ, D_V], F32, kind="ExternalOutput").ap()

    with tile.TileContext(nc) as tc:
        _emit(tc, io)
    nc.compile()
    return nc


_NC = None


def get_nc():
    global _NC
    if _NC is None:
        _NC = build_nc()
    return _NC


def make_in_maps(q, k, v, s_valid_mask, idt, qk_w, v_w, fc_w, fc_b, ln_g, ln_b):
    in_maps = []
    for b in range(B):
        in_maps.append({
            "q": np.ascontiguousarray(q[b], dtype=np.float32),
            "k": np.ascontiguousarray(k[b], dtype=np.float32),
            "v": np.ascontiguousarray(v[b], dtype=np.float32),
            "mask": np.ascontiguousarray(s_valid_mask[b], dtype=np.int32),
            "idt": np.ascontiguousarray(idt[b], dtype=np.float32),
            "qk_w": np.ascontiguousarray(qk_w, dtype=np.float32),
            "v_w": np.ascontiguousarray(v_w, dtype=np.float32),
            "fc_w": np.ascontiguousarray(fc_w, dtype=np.float32),
            "fc_b": np.ascontiguousarray(fc_b, dtype=np.float32),
            "ln_g": np.ascontiguousarray(ln_g, dtype=np.float32),
            "ln_b": np.ascontiguousarray(ln_b, dtype=np.float32),
        })
    return in_maps


def kernel(q, k, v, s_valid_mask, idt, qk_w, v_w, fc_w, fc_b, ln_g, ln_b,
           **run_kwargs):
    from concourse.bass_utils import run_bass_kernel_spmd

    nc = get_nc()
    in_maps = make_in_maps(q, k, v, s_valid_mask, idt,
                           qk_w, v_w, fc_w, fc_b, ln_g, ln_b)
    res = run_bass_kernel_spmd(nc, in_maps, core_ids=list(range(B)),
                               **run_kwargs)
    out = np.stack([res.results[b]["out"] for b in range(B)], axis=0)
    kernel.last_results = res
    return out.astype(np.float32)


# revision 18
# speedup vs baseline: 1.1366x; 1.1366x over previous
"""Trainium2 Bass kernel for nn_CrossAttention_17033840296537.

Full-input contract: kernel(**inputs) takes the unsharded tensors as in
reference.setup_inputs() and returns the full [8, 2048, 512] output.

Sharding: data-parallel over batch B=8 across the 8 NeuronCores (one
batch element per core). Weights are replicated.

Per-core design (fp8 DoubleRow for the dominant matmuls, f32 PSUM):
  prologue (everything SBUF-resident, no DRAM scratch):
    qk_w^T -> qkwT (bf16)
    q^T, k^T via PE transposes, projected to qhT/khT [512hd, 2048] bf16
    v cast to vtp fp8e4 pair tiles (lhsT for attn@V, 2 s-blocks each)
    M_h = wv_h^T @ fc_w[:,h]^T  [512c, 512o] bf16 per head -- merges the
        v-projection and the output fc into ONE matmul stage downstream.
  main loop, flat pipeline over 256 (j, h, si) pair-jobs:
    scores^T[s,q] = khT[h] slices^T @ qhT[h]   (K=64, bf16, tile_position
        by head parity)
    ptp = exp(scores*0.125 + mask_bias - C)  fp8e5 pair tile,
        UNNORMALIZED, C=2.5 keeps exp inside e5m2 range; the shift
        cancels exactly in the softmax division.
    t1[c,q]  += vtp^T ptp   (DoubleRow fp8: K=256 per instr, 2x rate)
    r[q]     += ones^T ptp  (softmax denominator, DoubleRow too)
    fc partial fp[q,o] = sum_cb t1s[cb]^T M_h[cb]  (bf16; fp8 here costs
        too much accuracy: rel-err 1.9e-2 vs the 2e-2 gate)
    facc[q,o] = fp * (1/r)[q] + facc   (ONE fused DVE op; 1/r arrives as
        a per-partition column via 4 tiny K=1 matmuls + reciprocal)
  fc/facc/epilogue work of iteration i is emitted interleaved into
  iteration i+1's score/t1 stream so the PE never drains.
  epilogue per j: += idt, LayerNorm (Sqrt on scalar, batched), -> out.
"""

import numpy as np

import concourse.bass as bass
import concourse.tile as tile
from concourse import mybir
from concourse.bass import ds
from concourse.masks import make_identity

F32 = mybir.dt.float32
BF = mybir.dt.bfloat16
E4 = mybir.dt.float8e4
E5 = mybir.dt.float8e5
I32 = mybir.dt.int32
AF = mybir.ActivationFunctionType
ALU = mybir.AluOpType
DRMODE = mybir.MatmulPerfMode.DoubleRow

B = 8
NQ = NS = 2048
DIM = 512          # input channel dim (DIM_K == DIM_V == 512)
N_HEAD = 8
D_K = 64
D_V = 512
HD = N_HEAD * D_V  # 4096 concat dim
P = 128
C_SHIFT = 2.5      # exp shift: max logit ~12.5, e5m2 max 57344=e^10.96


def _emit(tc: tile.TileContext, io: dict):
    nc = tc.nc
    q, k, v, mask, idt = io["q"], io["k"], io["v"], io["mask"], io["idt"]
    qk_w, v_w, fc_w = io["qk_w"], io["v_w"], io["fc_w"]
    fc_b, ln_g, ln_b = io["fc_b"], io["ln_g"], io["ln_b"]
    out = io["out"]
    rscr = nc.dram_tensor("rscr", [512], F32, kind="Internal").ap()

    cpool_cm = tc.tile_pool(name="cpool", bufs=1)
    rpool_cm = tc.tile_pool(name="rpool", bufs=1)
    cpool = cpool_cm.__enter__()
    rpool = rpool_cm.__enter__()

    # ---- constants ----
    ident = cpool.tile([P, P], F32, name="ident")
    make_identity(nc, ident)
    # 2x32 block of ones: DoubleRow stationary for the softmax-denominator
    # matmul (a [2,1]-shaped stationary fails the walrus ISA check; 32-wide
    # mirrors the minimum tile_size column group)
    ones_f = cpool.tile([P, 64], F32, name="ones_f")
    nc.vector.memset(ones_f, 1.0)
    ones2 = cpool.tile([P, 64], E4, name="ones2")
    nc.vector.tensor_copy(out=ones2, in_=ones_f)
    eps_t = cpool.tile([P, 1], F32, name="eps_t")
    nc.vector.memset(eps_t, 1e-5)

    def bcast_row(name, src):  # [512] dram -> [128, 512] sbuf (rows identical)
        bc = cpool.tile([P, D_V], F32, name=name + "_bc")
        src_b = bass.AP(tensor=src.tensor, offset=src.offset,
                        ap=[[0, P]] + list(src.ap))
        nc.gpsimd.dma_start(out=bc, in_=src_b)
        return bc

    fcb_bc = bcast_row("fcb", fc_b)
    lng_bc = bcast_row("lng", ln_g)
    lnb_bc = bcast_row("lnb", ln_b)

    mask_i = cpool.tile([P, 16], I32, name="mask_i")
    nc.gpsimd.dma_start(out=mask_i, in_=mask.rearrange("(a p) -> p a", p=P))
    mask_b = cpool.tile([P, 16], F32, name="mask_b")
    nc.vector.tensor_copy(out=mask_b, in_=mask_i)  # int32 -> f32 cast
    nc.scalar.mul(mask_b, mask_b, -10000.0)
    negc = cpool.tile([P, 1], F32, name="negc")
    nc.vector.memset(negc, -C_SHIFT)
    nc.vector.tensor_scalar(out=mask_b, in0=mask_b, scalar1=negc,
                            scalar2=None, op0=ALU.add)

    # ---- residents ----
    # vtp[si] holds s-blocks (2si, 2si+1) side by side as the DoubleRow
    # stationary operand for attn@V
    vtp = [rpool.tile([P, 2 * DIM], E4, name=f"vtp{si}") for si in range(8)]
    qhT = [rpool.tile([P, NQ], BF, name=f"qhT{mb}") for mb in range(4)]
    khT = [rpool.tile([P, NS], BF, name=f"khT{mb}") for mb in range(4)]
    Msb = [[rpool.tile([P, D_V], BF, name=f"M{h}_{cb}") for cb in range(4)]
           for h in range(N_HEAD)]
    facc = [rpool.tile([P, D_V], F32, name=f"facc{i}") for i in range(16)]

    # ================= prologue =================
    # DMA: 512-row blocks as single [128, 2048] rearranged transfers.
    # sync queue:   qk_w, q, k           (feeds the projection pipeline)
    # gpsimd queue: fc_w/v_w per head, v (weight/value path), then casts
    def blk(src, r0, cols=512, c0=0):
        # 512 DRAM rows (cols c0:c0+cols) -> [128 p, 4*cols] view where
        # element (p, a*cols + c) = src[r0 + a*128 + p, c0 + c]
        rstr = src.ap[0][0]
        return bass.AP(tensor=src.tensor,
                       offset=src.offset + r0 * rstr + c0,
                       ap=[[rstr, P], [P * rstr, 4], [1, cols]])

    with (
        tc.tile_pool(name="pstage", bufs=1) as pstage,
        tc.tile_pool(name="ppsum", bufs=1, space="PSUM") as pp,
    ):
        # ---- all DMA triggers up front ----
        # sync queue in priority order (q,k feed the PE first, then the
        # per-head weights) so transfers don't compete for HBM bandwidth;
        # v rides the gpsimd queue concurrently (small, needed mid-phase).
        qkw_stg = pstage.tile([P, 2048], F32, name="qkw_stg", tag="qld",
                              bufs=3)
        nc.sync.dma_start(out=qkw_stg, in_=blk(qk_w, 0))
        q0_stg = pstage.tile([P, 2048], F32, name="q0_stg", tag="qld",
                             bufs=3)
        nc.sync.dma_start(out=q0_stg, in_=blk(q, 0))
        qk_stg = [q0_stg]
        for src, sname in ((q, "q"), (k, "k")):
            for j2 in range(4):
                if src is q and j2 == 0:
                    continue
                st = pstage.tile([P, 2048], F32, name=f"{sname}stg{j2}",
                                 tag="qld", bufs=3)
                nc.sync.dma_start(out=st, in_=blk(src, j2 * 512))
                qk_stg.append(st)
        vstg = []
        for c4 in range(4):
            st = pstage.tile([P, 2048], F32, name=f"vstg{c4}", tag="vstg",
                             bufs=2)
            nc.gpsimd.dma_start(out=st, in_=blk(v, c4 * 512))
            vstg.append(st)
        wstg = []
        for h in range(N_HEAD):
            ft = pstage.tile([P, 2048], F32, name=f"fstg{h}", tag="wstg",
                             bufs=3)
            nc.sync.dma_start(out=ft, in_=blk(fc_w, 0, c0=h * 512))
            vw_raw = pstage.tile([P, 2048], F32, name=f"vwstg{h}",
                                 tag="wstg", bufs=3)
            nc.sync.dma_start(out=vw_raw, in_=blk(v_w, h * 512))
            wstg.append((ft, vw_raw))

        # v_w casts per head, split gpsimd/scalar so neither queue exceeds
        # the PE's ~5.1us per-head M-build pace
        vwb = []
        for h in range(N_HEAD):
            vws = []
            for i in range(4):
                vb = pstage.tile([P, 512], BF, name=f"vwb{h}_{i}",
                                 tag="vwb", bufs=5)
                eng = nc.gpsimd if i < 2 else nc.scalar
                if eng is nc.gpsimd:
                    nc.gpsimd.tensor_copy(out=vb,
                                          in_=wstg[h][1][:, ds(i * 512, 512)])
                else:
                    nc.scalar.copy(out=vb,
                                   in_=wstg[h][1][:, ds(i * 512, 512)])
                vws.append(vb)
            vwb.append(vws)

        # ---- qk_w^T -> qkwT bf16 ----
        qkwT = []
        for cb in range(4):
            tp = pp.tile([P, 512], F32, name=f"tpw{cb}", tag="tp", bufs=3)
            for rb in range(4):
                nc.tensor.transpose(tp[:, ds(rb * P, P)],
                                    qkw_stg[:, ds(rb * 512 + cb * P, P)],
                                    ident)
            qw = pstage.tile([P, 512], BF, name=f"qkwT{cb}", tag=f"qkwT{cb}")
            nc.vector.tensor_copy(out=qw, in_=tp)
            qkwT.append(qw)

        # ---- q, k: transpose + project -> qhT/khT bf16 (SBUF resident) ----
        # transposes run one chunk ahead of the projection waves so the
        # DVE qTc copies are always hidden behind PE work
        def emit_qkT(ci):
            stg = qk_stg[ci]
            qTc = []
            for cb in range(4):
                tp = pp.tile([P, 512], F32, name=f"tpq{ci}_{cb}",
                             tag="tp", bufs=3)
                for qb in range(4):
                    nc.tensor.transpose(
                        tp[:, ds(qb * P, P)],
                        stg[:, ds(qb * 512 + cb * P, P)], ident)
                qc = pstage.tile([P, 512], BF, name=f"qTc{ci}_{cb}",
                                 tag="qTc", bufs=8)
                nc.vector.tensor_copy(out=qc, in_=tp)
                qTc.append(qc)
            return qTc

        def emit_proj(ci, qTc):
            dstT = qhT if ci < 4 else khT
            j2 = ci % 4
            # cb-outer so each matmul wave depends on only one qTc copy
            prs = [pp.tile([P, 512], F32, name=f"pr{ci}_{mb}",
                           tag="pr", bufs=4) for mb in range(4)]
            for cb in range(4):
                for mb in range(4):
                    nc.tensor.matmul(prs[mb],
                                     lhsT=qkwT[cb][:, ds(mb * P, P)],
                                     rhs=qTc[cb],
                                     start=(cb == 0), stop=(cb == 3))
            for mb in range(4):
                nc.scalar.copy(out=dstT[mb][:, ds(j2 * 512, 512)],
                               in_=prs[mb])

        qTc_cur = emit_qkT(0)
        for ci in range(8):
            qTc_nxt = emit_qkT(ci + 1) if ci < 7 else None
            emit_proj(ci, qTc_cur)
            qTc_cur = qTc_nxt

        # ---- per-head merged projection M_h = wv_h^T @ fc_w[:,h]^T ----
        # fwT transposes run one head ahead of the M matmul waves
        def emit_fwT(h):
            fstg = wstg[h][0]
            fwT = []
            for db in range(4):
                tp = pp.tile([P, 512], F32, name=f"tpf{h}_{db}",
                             tag="tp", bufs=3)
                for rb in range(4):
                    nc.tensor.transpose(
                        tp[:, ds(rb * P, P)],
                        fstg[:, ds(rb * 512 + db * P, P)], ident)
                fw = pstage.tile([P, 512], BF, name=f"fwT{h}_{db}",
                                 tag="fwT", bufs=8)
                nc.vector.tensor_copy(out=fw, in_=tp)
                fwT.append(fw)
            return fwT

        def emit_M(h, fwT):
            # two v casts per head on DVE -- spreads them so vtp is ready
            # just before the main loop consumes it
            for half, sb in enumerate((2 * h, 2 * h + 1)):
                nc.vector.tensor_copy(
                    out=vtp[h][:, ds(half * 512, 512)],
                    in_=vstg[sb // 4][:, ds((sb % 4) * 512, 512)])
            # i-outer so each matmul wave depends on only one fwT copy
            prs = [pp.tile([P, 512], F32, name=f"prM{h}_{cb}",
                           tag="pr", bufs=4) for cb in range(4)]
            for i in range(4):
                for cb in range(4):
                    nc.tensor.matmul(prs[cb],
                                     lhsT=vwb[h][i][:, ds(cb * P, P)],
                                     rhs=fwT[i],
                                     start=(i == 0), stop=(i == 3))
            for cb in range(4):
                nc.scalar.copy(out=Msb[h][cb], in_=prs[cb])

        fwT_cur = emit_fwT(0)
        for h in range(N_HEAD):
            fwT_nxt = emit_fwT(h + 1) if h < 7 else None
            emit_M(h, fwT_cur)
            fwT_cur = fwT_nxt

    # ================= main =================
    with (
        tc.tile_pool(name="ms", bufs=1) as ms,
        tc.tile_pool(name="mp", bufs=1, space="PSUM") as mp,
    ):
        iters = [(j, h) for j in range(4) for h in range(8)]
        idt_tiles = {}   # j -> tile
        prev = None      # dict carrying previous iteration's state
        ep_pending = []  # j values whose epilogue is ready to emit

        def emit_idt_loads(j):
            it = ms.tile([P, 2048], F32, name=f"idt{j}", tag="idt", bufs=1)
            nc.sync.dma_start(out=it, in_=blk(idt, j * 512))
            idt_tiles[j] = it

        def emit_fc_group(pv, qb, idx):
            fpt = mp.tile([P, 512], F32, name=f"fp{idx}_{qb}", tag="fp",
                          bufs=1)
            for cb in range(4):
                nc.tensor.matmul(fpt,
                                 lhsT=pv["t1s"][cb][:, ds(qb * P, P)],
                                 rhs=Msb[pv["h"]][cb],
                                 start=(cb == 0), stop=(cb == 3))
            i16 = pv["j"] * 4 + qb
            in1 = fcb_bc if pv["h"] == 0 else facc[i16]
            nc.vector.scalar_tensor_tensor(out=facc[i16], in0=fpt,
                                           scalar=pv["rinv"][:, ds(qb, 1)],
                                           in1=in1,
                                           op0=ALU.mult, op1=ALU.add)
            if pv["h"] == 7 and qb == 3:
                ep_pending.append(pv["j"])

        def emit_epilogue(j):
            # residual + LayerNorm, in place on the facc tiles
            xts, mvs = [], []
            for qb in range(4):
                i16 = j * 4 + qb
                xt = facc[i16]
                nc.vector.tensor_add(xt, xt,
                                     idt_tiles[j][:, ds(qb * 512, 512)])
                st = ms.tile([P, 6], F32, name=f"st{i16}", tag="st", bufs=4)
                nc.vector.bn_stats(out=st, in_=xt)
                mv = ms.tile([P, 2], F32, name=f"mv{i16}", tag="mv", bufs=4)
                nc.vector.bn_aggr(out=mv, in_=st)
                xts.append(xt)
                mvs.append(mv)
            sds = []
            for qb in range(4):  # batched so the scalar engine swaps its
                i16 = j * 4 + qb  # activation table Exp->Sqrt only once
                sd = ms.tile([P, 1], F32, name=f"sd{i16}", tag="sd", bufs=4)
                nc.scalar.activation(sd, mvs[qb][:, 1:2], AF.Sqrt,
                                     bias=eps_t)
                sds.append(sd)
            rstds = []
            for qb in range(4):
                i16 = j * 4 + qb
                rstd = ms.tile([P, 1], F32, name=f"rstd{i16}", tag="rstd",
                               bufs=4)
                nc.vector.reciprocal(rstd, sds[qb])
                rstds.append(rstd)
            for qb in range(4):
                i16 = j * 4 + qb
                xt = xts[qb]
                nc.vector.tensor_scalar(out=xt, in0=xt,
                                        scalar1=mvs[qb][:, 0:1],
                                        scalar2=rstds[qb],
                                        op0=ALU.subtract, op1=ALU.mult)
                nc.vector.tensor_mul(xt, xt, lng_bc)
                nc.vector.tensor_add(xt, xt, lnb_bc)
                nc.sync.dma_start(out=out[ds(i16 * P, P), :], in_=xt)

        # ---- flat pipeline over 256 pair-jobs (32 iters x 8 si) ----
        state = {}  # per-iter psum tiles, created at si == 0

        def emit_pair(n):
            idx, si = n // 8, n % 8
            j, h = iters[idx]
            par = h % 2
            tnum = h // 2
            po = par * D_K
            tiles = []
            for sb in (2 * si, 2 * si + 1):
                sct = mp.tile([P, 512], F32, name=f"sc{idx}_{sb}",
                              tag="sc", bufs=2)
                nc.tensor.matmul(sct,
                                 lhsT=khT[tnum][po:po + D_K, ds(sb * P, P)],
                                 rhs=qhT[tnum][po:po + D_K, ds(j * 512, 512)],
                                 start=True, stop=True,
                                 tile_position=(po, 0))
                tiles.append(sct)
            return tiles

        def emit_exp(n, pair):
            idx, si = n // 8, n % 8
            ptp = ms.tile([P, 1024], E5, name=f"pt{idx}_{si}", tag="pt",
                          bufs=3)
            for half in range(2):
                sb = 2 * si + half
                nc.scalar.activation(ptp[:, ds(half * 512, 512)], pair[half],
                                     AF.Exp, bias=mask_b[:, ds(sb, 1)],
                                     scale=0.125)
            return ptp

        def emit_t1_dr(n, ptp):
            idx, si = n // 8, n % 8
            if si == 0:
                state["t1"] = mp.tile([P, 4 * 512], F32, name=f"t1_{idx}",
                                      tag="t1", bufs=1)
                state["r2a"] = mp.tile([32, 512], F32, name=f"r2a_{idx}",
                                       tag="r2a", bufs=1)
            t1, r2a = state["t1"], state["r2a"]
            rhs = ptp.rearrange("p (two n) -> p two n", two=2)
            for cb in range(4):
                lhsT = bass.AP(tensor=vtp[si].tensor,
                               offset=vtp[si].offset + cb * P,
                               ap=[vtp[si].ap[0], [512, 2], [1, P]])
                nc.tensor.matmul(t1[:, ds(cb * 512, 512)], lhsT=lhsT,
                                 rhs=rhs, start=(si == 0), stop=(si == 7),
                                 perf_mode=DRMODE)
            lones = bass.AP(tensor=ones2.tensor, offset=ones2.offset,
                            ap=[ones2.ap[0], [32, 2], [1, 32]])
            nc.tensor.matmul(r2a, lhsT=lones, rhs=rhs,
                             start=(si == 0), stop=(si == 7),
                             perf_mode=DRMODE)

        def emit_iter_end(idx):
            # softmax sums: bounce PSUM row -> DRAM -> per-partition column
            # on the idle gpsimd DMA queue, reciprocal on DVE. The PE never
            # touches the transpose. t1 -> SBUF bf16 casts all on DVE so the
            # scalar engine stays dedicated to exp.
            nonlocal prev
            j, h = iters[idx]
            t1, r2a = state["t1"], state["r2a"]
            rs = ms.tile([1, 512], F32, name=f"rs{idx}", tag="rs", bufs=2)
            nc.vector.tensor_copy(out=rs, in_=r2a[0:1, :])
            nc.gpsimd.dma_start(out=rscr, in_=rs)
            rcolt = ms.tile([P, 4], F32, name=f"rcol{idx}", tag="rcolT",
                            bufs=2)
            nc.gpsimd.dma_start(
                out=rcolt, in_=bass.AP(tensor=rscr.tensor, offset=rscr.offset,
                                       ap=[[1, P], [P, 4]]))
            rinv = ms.tile([P, 4], F32, name=f"rinv{idx}", tag="rinv", bufs=2)
            nc.vector.reciprocal(rinv, rcolt)
            t1s = [ms.tile([P, 512], BF, name=f"t1s{idx}_{cb}", tag="t1s",
                           bufs=8) for cb in range(4)]
            for cb in range(4):
                nc.vector.tensor_copy(out=t1s[cb], in_=t1[:, ds(cb * 512, 512)])
            prev = {"j": j, "h": h, "t1s": t1s, "rinv": rinv}

        def emit_extras(n):
            idx, si = n // 8, n % 8
            j, h = iters[idx]
            if prev is not None and 1 <= si <= 4:
                emit_fc_group(prev, si - 1, idx)
            if si == 5:
                if h == 5:
                    emit_idt_loads(j)
                if ep_pending and h >= 1:
                    emit_epilogue(ep_pending.pop(0))

        # Two-pair-deep exp pipeline: the scalar engine's exp (~0.7us per
        # half) gets two full PE steps of slack, so the score matmuls'
        # WAR on the sc PSUM buffers (bufs=2) never stalls the PE.
        NJOBS = 256
        ptps = {}
        for m in range(2):
            ptps[m] = emit_exp(m, emit_pair(m))
        for n in range(NJOBS):
            if n + 2 < NJOBS:
                pair_nxt = emit_pair(n + 2)
            emit_t1_dr(n, ptps.pop(n))
            if n % 8 == 7:
                emit_iter_end(n // 8)
            if n + 2 < NJOBS:
                ptps[n + 2] = emit_exp(n + 2, pair_nxt)
            emit_extras(n)

        # drain the last iteration + epilogue for j=3
        for qb in range(4):
            emit_fc_group(prev, qb, 32)
        while ep_pending:
            emit_epilogue(ep_pending.pop(0))

    rpool_cm.__exit__(None, None, None)
    cpool_cm.__exit__(None, None, None)


def build_nc():
    from concourse import bacc
    nc = bacc.Bacc("TRN2", target_bir_lowering=False, debug=False)
    io = {}
    io["q"] = nc.dram_tensor("q", [NQ, DIM], F32, kind="ExternalInput").ap()
    io["k"] = nc.dram_tensor("k", [NS, DIM], F32, kind="ExternalInput").ap()
    io["v"] = nc.dram_tensor("v", [NS, DIM], F32, kind="ExternalInput").ap()
    io["mask"] = nc.dram_tensor("mask", [NS], I32, kind="ExternalInput").ap()
    io["idt"] = nc.dram_tensor("idt", [NQ, D_V], F32, kind="ExternalInput").ap()
    io["qk_w"] = nc.dram_tensor("qk_w", [512, DIM], F32, kind="ExternalInput").ap()
    io["v_w"] = nc.dram_tensor("v_w", [HD, DIM], F32, kind="ExternalInput").ap()
    io["fc_w"] = nc.dram_tensor("fc_w", [D_V, HD], F32, kind="ExternalInput").ap()
    io["fc_b"] = nc.dram_tensor("fc_b", [D_V], F32, kind="ExternalInput").ap()
    io["ln_g"] = nc.dram_tensor("ln_g", [D_V], F32, kind="ExternalInput").ap()
    io["ln_b"] = nc.dram_tensor("ln_b", [D_V], F32, kind="ExternalInput").ap()
    io["out"] = nc.dram_tensor("out", [NQ, D_V], F32, kind="ExternalOutput").ap()

    with tile.TileContext(nc) as tc:
        _emit(tc, io)
    nc.compile()
    return nc


_NC = None


def get_nc():
    global _NC
    if _NC is None:
        _NC = build_nc()
    return _NC


def make_in_maps(q, k, v, s_valid_mask, idt, qk_w, v_w, fc_w, fc_b, ln_g, ln_b):
    in_maps = []
    for b in range(B):
        in_maps.append({
            "q": np.ascontiguousarray(q[b], dtype=np.float32),
            "k": np.ascontiguousarray(k[b], dtype=np.float32),
            "v": np.ascontiguousarray(v[b], dtype=np.float32),
            "mask": np.ascontiguousarray(s_valid_mask[b], dtype=np.int32),
            "idt": np.ascontiguousarray(idt[b], dtype=np.float32),
            "qk_w": np.ascontiguousarray(qk_w, dtype=np.float32),
            "v_w": np.ascontiguousarray(v_w, dtype=np.float32),
            "fc_w": np.ascontiguousarray(fc_w, dtype=np.float32),
            "fc_b": np.ascontiguousarray(fc_b, dtype=np.float32),
            "ln_g": np.ascontiguousarray(ln_g, dtype=np.float32),
            "ln_b": np.ascontiguousarray(ln_b, dtype=np.float32),
        })
    return in_maps


def kernel(q, k, v, s_valid_mask, idt, qk_w, v_w, fc_w, fc_b, ln_g, ln_b,
           **run_kwargs):
    from concourse.bass_utils import run_bass_kernel_spmd

    nc = get_nc()
    in_maps = make_in_maps(q, k, v, s_valid_mask, idt,
                           qk_w, v_w, fc_w, fc_b, ln_g, ln_b)
    res = run_bass_kernel_spmd(nc, in_maps, core_ids=list(range(B)),
                               **run_kwargs)
    out = np.stack([res.results[b]["out"] for b in range(B)], axis=0)
    kernel.last_results = res
    return out.astype(np.float32)
